# revision 1
# baseline (speedup 1.0000x reference)
"""Trainium2 Bass kernel for nn_MixedOp_35562329211102.

Computes FM[b,c] = expm( sum_o weights[o] * logm( W[o,c]^T x[b,c] W[o,c] ) )
for x: [256,16,64,64] SPD, W: [6,16,64,32], weights: [6] (simplex).

Algorithm (matmul/elementwise only, no eigendecomposition):
  logm via a "W-only inverse-scaling" iteration: A = Y/theta, W0 = A,
  W_{j+1} = W_j * q_j(W_j)^2 with q_j(w) = a_j + b_j w chosen so the
  spectrum [1.7e-4, 0.96] is driven into [0.38, 1].  log(A) is then a
  fixed linear combination (global minimax fit, sup err ~5e-4) of the
  intermediates {I, W_j, G_j = W_j q_j(W_j), W_f, W_f^2, W_f^3, W_f^4}.
  15 matmuls (32x32) per logm.
  expm via scaling-squaring: X = M/8, degree-8 Taylor (Paterson-
  Stockmeyer), then 3 squarings.  8 matmuls per expm.

Sharding: data-parallel over batch B across 8 cores (32 batches/core).
"""

import numpy as np

import concourse.bass as bass
from concourse import bacc
import concourse.mybir as mybir
from concourse.bass import AP
from concourse.tile import TileContext

FP = mybir.dt.float32
AOP = mybir.AluOpType

THETA = 9.0
LOGTHETA = 2.1972245773
ITERS = [
    (1.97108588, -1.13452036),
    (1.92678581, -1.0597322),
    (1.92678489, -1.05973169),
    (1.92678489, -1.05973169),
    (1.92678489, -1.05973169),
    (1.92678489, -1.05973169),
]
NIT = len(ITERS)
COEF = {
    'one': -10.50386520,
    'W0': 2.69748291, 'G0': -1.48453522,
    'W1': 3.00410138, 'G1': -1.52398907,
    'W2': 3.03186360, 'G2': -1.53628015,
    'W3': 3.03982436, 'G3': -1.54172997,
    'W4': 3.04361117, 'G4': -1.54522990,
    'W5': 3.04648630, 'G5': -1.54683948,
    'Wf': 6.88600636, 'P2': -7.62809900, 'P3': 5.13819165, 'P4': -1.40122234,
}
EXPC = [1.0, 1.0, 0.5, 1.0 / 6, 1.0 / 24, 1.0 / 120, 1.0 / 720, 1.0 / 5040,
        1.0 / 40320]

C, O, D, DIN = 16, 6, 32, 64
NCORES = 8

WT_KINDS = [f'W{j}' for j in range(NIT)] + ['Wf'] \
    + [f'G{j}' for j in range(NIT)] + ['P2', 'P3', 'P4']
WT_NCOL = len(WT_KINDS) * O


def host_wtab(weights: np.ndarray) -> np.ndarray:
    """[128, WT_NCOL] per-partition scalar table: w[o]/8 * coef (W0 also /theta)."""
    w8 = weights.astype(np.float64) / 8.0
    cols = []
    for k in WT_KINDS:
        s = COEF[k] / THETA if k == 'W0' else COEF[k]
        cols.append(w8 * s)
    row = np.concatenate(cols)
    return np.tile(row[None, :], (128, 1)).astype(np.float32)


def host_idt() -> np.ndarray:
    """[128, 32]: 4 stacked 32x32 identities."""
    return np.tile(np.eye(D, dtype=np.float32), (4, 1))


def _bc(t, nblk):
    """broadcast a [128, D] tile AP over nblk column blocks -> [128, nblk, D]."""
    a = t[:, :]
    return AP(a.tensor, a.offset, [list(a.ap[0]), [0, nblk], [1, D]])


def _blk(ap, nblk):
    """view a [128, nblk*D] AP as [128, nblk, D]."""
    return ap.rearrange("p (n j) -> p n j", n=nblk)


def build_nc(b_loc=32, bchunk=8, replicate=1):
    nchunk = b_loc // bchunk
    nb = bchunk * D          # stage2 N per (o,c)
    ncols = 4 * bchunk * D   # X / og tile width
    nblk = 4 * bchunk        # 32x32 col-blocks per og tile

    nc = bacc.Bacc("TRN2")
    x = nc.dram_tensor("x", [b_loc, C, DIN, DIN], FP, kind="ExternalInput")
    Wt = nc.dram_tensor("W", [O, C, DIN, D], FP, kind="ExternalInput")
    wtab_d = nc.dram_tensor("wtab", [128, WT_NCOL], FP, kind="ExternalInput")
    idt_d = nc.dram_tensor("idt", [128, D], FP, kind="ExternalInput")
    out = nc.dram_tensor("out", [b_loc, C, D, D], FP, kind="ExternalOutput")

    with TileContext(nc) as tc, (
        tc.tile_pool(name="consts", bufs=1)) as consts, (
        tc.tile_pool(name="xp", bufs=4)) as xp, (
        tc.tile_pool(name="vp", bufs=2)) as vp, (
        tc.tile_pool(name="wog", bufs=10)) as wogp, (
        tc.tile_pool(name="tp", bufs=4)) as tp, (
        tc.tile_pool(name="gp", bufs=4)) as gp, (
        tc.tile_pool(name="ct", bufs=7)) as ctp, (
        tc.tile_pool(name="outp", bufs=2)) as outp, (
        tc.tile_pool(name="xaccp", bufs=2)) as xaccp, (
        tc.tile_pool(name="s1ps", bufs=1, space="PSUM")) as s1psp, (
        tc.tile_pool(name="s2ps", bufs=1, space="PSUM")) as s2psp, (
        tc.tile_pool(name="wkps", bufs=3, space="PSUM")) as wkps:

        # ---- constants ----
        w1t = []
        for cp in range(C // 2):
            t = consts.tile([128, O * D], FP, tag=f"w1_{cp}")
            for e in range(2):
                dst = t[64 * e:64 * (e + 1), :].rearrange("p (o j) -> p o j", o=O)
                src = Wt[:, 2 * cp + e, :, :].rearrange("o p j -> p o j")
                nc.sync.dma_start(dst, src)
            w1t.append(t)
        wtab = consts.tile([128, WT_NCOL], FP, tag="wtab", name="wtab")
        nc.sync.dma_start(wtab[:, :], wtab_d[:, :])
        idt = consts.tile([128, D], FP, tag="idt", name="idt")
        nc.sync.dma_start(idt[:, :], idt_d[:, :])
        aid = []
        for j, (a, b) in enumerate(ITERS):
            t = consts.tile([128, D], FP, tag=f"aid{j}")
            nc.vector.tensor_scalar_mul(t[:, :], idt[:, :], float(a))
            aid.append(t)
        cid = {}
        for k in (0, 3, 6):
            t = consts.tile([128, D], FP, tag=f"cid{k}")
            nc.vector.tensor_scalar_mul(t[:, :], idt[:, :], float(EXPC[k]))
            cid[k] = t

        def wap(kind, o):
            i = WT_KINDS.index(kind) * O + o
            return wtab[:, i:i + 1]

        def mmwave(dst, lhs, rhs, start=True, stop=True):
            for cb in range(nblk):
                for i in range(4):
                    sl = slice(i * D, (i + 1) * D)
                    cs = slice(cb * D, (cb + 1) * D)
                    nc.tensor.matmul(dst[sl, cs], lhs[sl, cs], rhs[sl, cs],
                                     start=start, stop=stop,
                                     tile_position=(i * D, i * D))

        for _rep in range(replicate):
          for ch in range(nchunk):
            if True:
                Xps = xaccp.tile([128, ncols], FP, tag="xacc", name="xacc")
                nc.vector.memset(Xps[:, :], 0.0)
                wog = [wogp.tile([128, ncols], FP, tag="wog", name="wog") for _ in range(O)]

                # ===== phase A: BiMap =====
                if True:
                    for q in range(4):
                        vt = vp.tile([128, 2 * O * nb], FP, tag="v", name="v")
                        for cp in (2 * q, 2 * q + 1):
                            e = cp % 2
                            for bb in range(bchunk):
                                b = ch * bchunk + bb
                                xt = xp.tile([128, DIN], FP, tag="xt", name="xt")
                                xsrc = x[b, 2 * cp:2 * cp + 2].rearrange(
                                    "c p j -> (c p) j")
                                nc.sync.dma_start(xt[:, :], xsrc)
                                ps1 = s1psp.tile([128, O * D], FP, tag="s1", name="s1")
                                nc.tensor.matmul(ps1[0:64, :], xt[0:64, :],
                                                 w1t[cp][0:64, :],
                                                 tile_position=(0, 0))
                                nc.tensor.matmul(ps1[64:128, :], xt[64:128, :],
                                                 w1t[cp][64:128, :],
                                                 tile_position=(64, 64))
                                # scatter V into o-major layout
                                src = ps1[:, :].rearrange("p (o j) -> p o j", o=O)
                                va = vt[:, :]
                                dst = AP(va.tensor,
                                         va.offset + e * O * nb + bb * D,
                                         [list(va.ap[0]), [nb, O], [1, D]])
                                nc.vector.tensor_copy(dst, src)
                        for o in range(O):
                            ps2 = s2psp.tile([128, nb], FP, tag="s2", name="s2")
                            for cp in (2 * q, 2 * q + 1):
                                e = cp % 2
                                for par in range(2):
                                    r = 2 * e + par
                                    nc.tensor.matmul(
                                        ps2[r * D:(r + 1) * D, :],
                                        w1t[cp][par * 64:(par + 1) * 64,
                                                o * D:(o + 1) * D],
                                        vt[par * 64:(par + 1) * 64,
                                           e * O * nb + o * nb:
                                           e * O * nb + (o + 1) * nb],
                                        tile_position=(par * 64, r * D))
                            # evacuate Y -> Wcur (x 1/theta), accumulate W0 term
                            nc.vector.tensor_scalar_mul(
                                wog[o][:, q * nb:(q + 1) * nb],
                                ps2[:, :], 1.0 / THETA)
                            nc.vector.scalar_tensor_tensor(
                                Xps[:, q * nb:(q + 1) * nb],
                                ps2[:, :], wap('W0', o),
                                Xps[:, q * nb:(q + 1) * nb],
                                op0=AOP.mult, op1=AOP.add)

                # ===== phases B+C =====
                if True:
                    # phase B: log iterations (og pairs interleaved for
                    # PE/DVE overlap); weighted accumulation on gpsimd
                    for op in range(0, O, 2):
                        wcur = [wog[op], wog[op + 1]]
                        for j in range(NIT):
                            a, b = ITERS[j]
                            tt = []
                            for m in range(2):
                                t = tp.tile([128, ncols], FP, tag="t", name="t")
                                nc.vector.scalar_tensor_tensor(
                                    _blk(t[:, :], nblk), _blk(wcur[m][:, :], nblk),
                                    float(b), _bc(aid[j], nblk),
                                    op0=AOP.mult, op1=AOP.add)
                                tt.append(t)
                            gps = []
                            for m in range(2):
                                ps = wkps.tile([128, ncols], FP, tag="wk", name="wk")
                                mmwave(ps, tt[m], wcur[m])
                                gps.append(ps)
                            gt = []
                            for m in range(2):
                                g = gp.tile([128, ncols], FP, tag="g", name="g")
                                nc.vector.tensor_copy(g[:, :], gps[m][:, :])
                                gt.append(g)
                            for m in range(2):
                                nc.vector.scalar_tensor_tensor(
                                    Xps[:, :], gt[m][:, :], wap(f'G{j}', op + m),
                                    Xps[:, :], op0=AOP.mult, op1=AOP.add)
                            kind = f'W{j + 1}' if j + 1 < NIT else 'Wf'
                            for m in range(2):
                                ps = wkps.tile([128, ncols], FP, tag="wk", name="wk")
                                mmwave(ps, tt[m], gt[m])
                                wnew = wogp.tile([128, ncols], FP, tag="wog",
                                                 name="wog")
                                nc.scalar.copy(wnew[:, :], ps[:, :])
                                nc.vector.scalar_tensor_tensor(
                                    Xps[:, :], wnew[:, :], wap(kind, op + m),
                                    Xps[:, :], op0=AOP.mult, op1=AOP.add)
                                wcur[m] = wnew
                        # tail powers of Wf
                        p2t = []
                        for m in range(2):
                            ps = wkps.tile([128, ncols], FP, tag="wk", name="wk")
                            mmwave(ps, wcur[m], wcur[m])
                            p2 = gp.tile([128, ncols], FP, tag="g", name="g")
                            nc.vector.tensor_copy(p2[:, :], ps[:, :])
                            nc.vector.scalar_tensor_tensor(
                                Xps[:, :], p2[:, :], wap('P2', op + m),
                                Xps[:, :], op0=AOP.mult, op1=AOP.add)
                            p2t.append(p2)
                        for m in range(2):
                            ps = wkps.tile([128, ncols], FP, tag="wk", name="wk")
                            mmwave(ps, p2t[m], wcur[m])
                            nc.vector.scalar_tensor_tensor(
                                Xps[:, :], ps[:, :], wap('P3', op + m),
                                Xps[:, :], op0=AOP.mult, op1=AOP.add)
                        for m in range(2):
                            ps = wkps.tile([128, ncols], FP, tag="wk", name="wk")
                            mmwave(ps, p2t[m], p2t[m])
                            nc.vector.scalar_tensor_tensor(
                                Xps[:, :], ps[:, :], wap('P4', op + m),
                                Xps[:, :], op0=AOP.mult, op1=AOP.add)

                    # const term: X += ((COEF.one + LOGTHETA)/8) * I
                    nc.vector.scalar_tensor_tensor(
                        _blk(Xps[:, :], nblk), _bc(idt, nblk),
                        float((COEF['one'] + LOGTHETA) / 8.0),
                        _blk(Xps[:, :], nblk), op0=AOP.mult, op1=AOP.add)

                    # phase C: expm
                    xs = ctp.tile([128, ncols], FP, tag="ctmp", name="ctmp")
                    nc.vector.tensor_copy(xs[:, :], Xps[:, :])
                    x2ps = wkps.tile([128, ncols], FP, tag="wk", name="wk")
                    mmwave(x2ps, xs, xs)
                    x2t = ctp.tile([128, ncols], FP, tag="ctmp", name="ctmp")
                    nc.vector.tensor_copy(x2t[:, :], x2ps[:, :])
                    x3ps = wkps.tile([128, ncols], FP, tag="wk", name="wk")
                    mmwave(x3ps, x2t, xs)
                    x3t = ctp.tile([128, ncols], FP, tag="ctmp", name="ctmp")
                    nc.vector.tensor_copy(x3t[:, :], x3ps[:, :])
                    h1 = ctp.tile([128, ncols], FP, tag="ctmp", name="ctmp")
                    nc.vector.scalar_tensor_tensor(
                        _blk(h1[:, :], nblk), _blk(xs[:, :], nblk),
                        float(EXPC[4]), _bc(cid[3], nblk),
                        op0=AOP.mult, op1=AOP.add)
                    nc.vector.scalar_tensor_tensor(
                        h1[:, :], x2t[:, :], float(EXPC[5]), h1[:, :],
                        op0=AOP.mult, op1=AOP.add)
                    nc.vector.scalar_tensor_tensor(
                        h1[:, :], x3t[:, :], float(EXPC[6]), h1[:, :],
                        op0=AOP.mult, op1=AOP.add)
                    plow = ctp.tile([128, ncols], FP, tag="ctmp", name="ctmp")
                    nc.vector.scalar_tensor_tensor(
                        _blk(plow[:, :], nblk), _blk(xs[:, :], nblk),
                        float(EXPC[1]), _bc(cid[0], nblk),
                        op0=AOP.mult, op1=AOP.add)
                    nc.vector.scalar_tensor_tensor(
                        plow[:, :], x2t[:, :], float(EXPC[2]), plow[:, :],
                        op0=AOP.mult, op1=AOP.add)
                    ppps = wkps.tile([128, ncols], FP, tag="wk", name="wk")
                    mmwave(ppps, x3t, h1)
                    e0 = ctp.tile([128, ncols], FP, tag="ctmp", name="ctmp")
                    nc.vector.scalar_tensor_tensor(
                        e0[:, :], ppps[:, :], 1.0, plow[:, :],
                        op0=AOP.mult, op1=AOP.add)
                    e1ps = wkps.tile([128, ncols], FP, tag="wk", name="wk")
                    mmwave(e1ps, e0, e0)
                    e1 = ctp.tile([128, ncols], FP, tag="ctmp", name="ctmp")
                    nc.vector.tensor_copy(e1[:, :], e1ps[:, :])
                    e2ps = wkps.tile([128, ncols], FP, tag="wk", name="wk")
                    mmwave(e2ps, e1, e1)
                    e2 = ctp.tile([128, ncols], FP, tag="ctmp", name="ctmp")
                    nc.vector.tensor_copy(e2[:, :], e2ps[:, :])
                    e3ps = wkps.tile([128, ncols], FP, tag="wk", name="wk")
                    mmwave(e3ps, e2, e2)
                    outt = outp.tile([128, ncols], FP, tag="outt", name="outt")
                    nc.vector.tensor_copy(outt[:, :], e3ps[:, :])
                    # dst AP dims match src iteration order: (r,i | b,j), per q
                    oa = out[:, :, :, :]
                    for q in range(4):
                        dst = AP(oa.tensor,
                                 ch * bchunk * C * D * D + q * 4 * D * D,
                                 [[D * D, 4], [D, D],
                                  [C * D * D, bchunk], [1, D]])
                        src = outt[:, q * nb:(q + 1) * nb].rearrange(
                            "p (b j) -> p b j", b=bchunk)
                        nc.sync.dma_start(dst, src)
    return nc


_NC_CACHE = {}


def kernel(x: np.ndarray, W: np.ndarray, weights: np.ndarray) -> np.ndarray:
    from concourse.bass_utils import run_bass_kernel_spmd
    B = x.shape[0]
    b_loc = B // NCORES
    key = (b_loc,)
    if key not in _NC_CACHE:
        nc0 = build_nc(b_loc=b_loc, bchunk=8)
        nc0.finalize()
        _NC_CACHE[key] = nc0
    nc = _NC_CACHE[key]
    wtab = host_wtab(np.asarray(weights))
    idt = host_idt()
    in_maps = [
        {"x": np.ascontiguousarray(x[i * b_loc:(i + 1) * b_loc]).astype(np.float32),
         "W": np.ascontiguousarray(W).astype(np.float32),
         "wtab": wtab, "idt": idt}
        for i in range(NCORES)
    ]
    res = run_bass_kernel_spmd(nc, in_maps, core_ids=list(range(NCORES)))
    return np.concatenate([r["out"] for r in res.results], axis=0)

